# revision 8
# baseline (speedup 1.0000x reference)
"""Conv2D 3x3 (stride 1, pad 1) NCHW on 8 TRN2 NeuronCores.

x: (32, 128, 56, 56) f32, weight: (256, 128, 3, 3) OIHW, bias: (256,)
out: (32, 256, 56, 56) f32.

Strategy: data-parallel over batch (4 images per core, weight/bias
replicated). Per image, the padded input (128 x 58 x 58) lives in SBUF with
C_in=128 on partitions. The 3x3 conv is 9 shifted [128x128] @ [128x448]
matmuls accumulated in PSUM (output tile = 2 co-tiles x 8 rows x 56 cols),
using float32r operands (full PE rate at ~1.5e-4 rel err). Bias is added on
the vector engine while evacuating PSUM -> SBUF, then DMA to HBM.
"""

import numpy as np

import concourse.tile as tile
from concourse import bacc, mybir
from concourse.bass_utils import run_bass_kernel_spmd

N_CORES = 8
N_BATCH = 32
N_PER_CORE = N_BATCH // N_CORES  # 4
C_IN, C_OUT, H, W = 128, 256, 56, 56
HP, WP = H + 2, W + 2  # 58 (zero-padded)
ROWS = 8  # output rows per PSUM tile
N_RTILES = H // ROWS  # 7
NFREE = ROWS * W  # 448 <= 512 (one PSUM bank, f32r full-rate needs >= 256)
N_CT = C_OUT // 128  # 2 co-tiles


def build_nc(n_imgs=N_PER_CORE):
    f32 = mybir.dt.float32
    f32r = mybir.dt.float32r
    nc = bacc.Bacc("TRN2", target_bir_lowering=False, debug=False)
    x = nc.dram_tensor("x", [n_imgs, C_IN, H, W], f32r, kind="ExternalInput")
    w = nc.dram_tensor("w", [C_IN, 9 * C_OUT], f32r, kind="ExternalInput")
    b = nc.dram_tensor("b", [C_IN, N_CT], f32, kind="ExternalInput")
    z = nc.dram_tensor("z", [C_IN, WP], f32r, kind="ExternalInput")  # zeros
    out = nc.dram_tensor("out", [n_imgs, C_OUT, H * W], f32, kind="ExternalOutput")

    with tile.TileContext(nc) as tc:
        with tc.tile_pool(name="wpool", bufs=1) as wpool, \
             tc.tile_pool(name="xpool", bufs=1) as xpool, \
             tc.tile_pool(name="opool", bufs=4) as opool, \
             tc.tile_pool(name="pspool", bufs=4, space="PSUM") as pspool:
            w_sb = wpool.tile([C_IN, 9 * C_OUT], f32r)
            nc.sync.dma_start(w_sb[:], w[:])
            b_sb = wpool.tile([C_IN, N_CT], f32)
            nc.sync.dma_start(b_sb[:], b[:])

            # two persistent padded-image buffers (manual ping/pong); the
            # one-pixel border is zeroed once by DMA (walrus requires f32r
            # matmul inputs to be produced as f32r; memset can't encode it)
            xps = []
            for i in range(2):
                xp = xpool.tile([C_IN, HP, WP], f32r, name=f"xp{i}", tag=f"xp{i}")
                nc.sync.dma_start(xp[:, 0, :], z[:])
                nc.sync.dma_start(xp[:, HP - 1, :], z[:])
                nc.sync.dma_start(xp[:, 1:HP - 1, 0], z[:, :H])
                nc.sync.dma_start(xp[:, 1:HP - 1, WP - 1], z[:, :H])
                xps.append(xp)

            for n in range(n_imgs):
                xp = xps[n % 2]
                # interior loaded in 8-row chunks (parallel DMA queues)
                for a in range(0, H, ROWS):
                    nc.sync.dma_start(
                        xp[:, 1 + a:1 + a + ROWS, 1:1 + W],
                        x[n, :, a:a + ROWS, :],
                    )
                for r in range(N_RTILES):
                    for ct in range(N_CT):
                        pt = pspool.tile([128, NFREE], f32, tag="pt")
                        for tap in range(9):
                            kh, kw = tap // 3, tap % 3
                            c0 = tap * C_OUT + ct * 128
                            nc.tensor.matmul(
                                pt[:],
                                w_sb[:, c0:c0 + 128],
                                xp[:, r * ROWS + kh:r * ROWS + kh + ROWS, kw:kw + W],
                                start=(tap == 0),
                                stop=(tap == 8),
                            )
                        ot = opool.tile([128, NFREE], f32, tag="ot")
                        nc.vector.tensor_scalar_add(ot[:], pt[:], b_sb[:, ct:ct + 1])
                        nc.sync.dma_start(
                            out[n, ct * 128:(ct + 1) * 128, r * NFREE:(r + 1) * NFREE],
                            ot[:],
                        )
    nc.compile()
    return nc


def _host_prep(x, weight, bias):
    x = np.ascontiguousarray(x, dtype=np.float32)
    # weight OIHW -> [ci, (kh kw co)] so each lhsT tile is a contiguous slice
    w_host = np.ascontiguousarray(
        weight.astype(np.float32).transpose(1, 2, 3, 0).reshape(C_IN, 9 * C_OUT)
    )
    # bias[co] -> [co % 128, co // 128]
    b_host = np.ascontiguousarray(bias.astype(np.float32).reshape(N_CT, 128).T)
    return x, w_host, b_host


def kernel(x, weight, bias, _trace=False):
    x, w_host, b_host = _host_prep(x, weight, bias)
    nc = build_nc()
    z_host = np.zeros((C_IN, WP), dtype=np.float32)
    in_maps = [
        {"x": x[i * N_PER_CORE:(i + 1) * N_PER_CORE], "w": w_host, "b": b_host,
         "z": z_host}
        for i in range(N_CORES)
    ]
    res = run_bass_kernel_spmd(nc, in_maps, core_ids=list(range(N_CORES)), trace=_trace)
    out = np.concatenate(
        [res.results[i]["out"].reshape(N_PER_CORE, C_OUT, H, W) for i in range(N_CORES)],
        axis=0,
    )
    if _trace:
        return out, res
    return out


# revision 11
# speedup vs baseline: 1.1141x; 1.1141x over previous
"""Conv2D 3x3 (stride 1, pad 1) NCHW on 8 TRN2 NeuronCores.

x: (32, 128, 56, 56) f32, weight: (256, 128, 3, 3) OIHW, bias: (256,)
out: (32, 256, 56, 56) f32.

Strategy: data-parallel over batch (4 images per core, weight/bias
replicated). The input is zero-padded to 58x58 on the host, so each padded
image lives in SBUF with C_in=128 on partitions and needs no on-device
border handling. The 3x3 conv is 9 shifted [128x128] @ [128x448] matmuls
accumulated in PSUM (output tile = 8 rows x 56 cols per co-tile), using
float32r operands (full PE rate, ~1.5e-4 rel err). Bias is added on the
vector engine while evacuating PSUM -> SBUF, then DMA to HBM.
"""

import numpy as np

import concourse.tile as tile
from concourse import bacc, mybir
from concourse.bass_utils import run_bass_kernel_spmd

N_CORES = 8
N_BATCH = 32
N_PER_CORE = N_BATCH // N_CORES  # 4
C_IN, C_OUT, H, W = 128, 256, 56, 56
HP, WP = H + 2, W + 2  # 58 (zero-padded on host)
ROWS = 8  # output rows per PSUM tile
N_RTILES = H // ROWS  # 7
NFREE = ROWS * W  # 448 <= 512 (one PSUM bank; f32r full rate needs >= 256)
N_CT = C_OUT // 128  # 2 co-tiles


def build_nc(n_imgs=N_PER_CORE):
    f32 = mybir.dt.float32
    f32r = mybir.dt.float32r
    nc = bacc.Bacc("TRN2", target_bir_lowering=False, debug=False)
    x = nc.dram_tensor("x", [n_imgs, C_IN, HP, WP], f32r, kind="ExternalInput")
    w = nc.dram_tensor("w", [C_IN, 9 * C_OUT], f32r, kind="ExternalInput")
    b = nc.dram_tensor("b", [C_IN, N_CT], f32, kind="ExternalInput")
    out = nc.dram_tensor("out", [n_imgs, C_OUT, H * W], f32, kind="ExternalOutput")

    with tile.TileContext(nc) as tc:
        with tc.tile_pool(name="wpool", bufs=1) as wpool, \
             tc.tile_pool(name="xpool", bufs=2) as xpool, \
             tc.tile_pool(name="opool", bufs=4) as opool, \
             tc.tile_pool(name="pspool", bufs=4, space="PSUM") as pspool:
            w_sb = wpool.tile([C_IN, 9 * C_OUT], f32r)
            for tap in range(9):
                # split by tap: parallel DMA queues + subtile deps let the
                # first matmuls start long before the full weight arrives
                sl = slice(tap * C_OUT, (tap + 1) * C_OUT)
                nc.sync.dma_start(w_sb[:, sl], w[:, sl])
            b_sb = wpool.tile([C_IN, N_CT], f32)
            nc.sync.dma_start(b_sb[:], b[:])

            for n in range(n_imgs):
                xp = xpool.tile([C_IN, HP, WP], f32r, tag="xp")
                # padded image loaded in 8-row chunks (parallel DMA queues)
                for a in range(0, HP, ROWS):
                    e = min(a + ROWS, HP)
                    nc.sync.dma_start(xp[:, a:e, :], x[n, :, a:e, :])
                for r in range(N_RTILES):
                    for ct in range(N_CT):
                        pt = pspool.tile([128, NFREE], f32, tag="pt")
                        for tap in range(9):
                            kh, kw = tap // 3, tap % 3
                            c0 = tap * C_OUT + ct * 128
                            nc.tensor.matmul(
                                pt[:],
                                w_sb[:, c0:c0 + 128],
                                xp[:, r * ROWS + kh:r * ROWS + kh + ROWS, kw:kw + W],
                                start=(tap == 0),
                                stop=(tap == 8),
                            )
                        ot = opool.tile([128, NFREE], f32, tag="ot")
                        nc.vector.tensor_scalar_add(ot[:], pt[:], b_sb[:, ct:ct + 1])
                        # output split in two DMAs to shrink the drain tail
                        half = NFREE // 2
                        for hh in range(2):
                            nc.sync.dma_start(
                                out[n, ct * 128:(ct + 1) * 128,
                                    r * NFREE + hh * half:r * NFREE + (hh + 1) * half],
                                ot[:, hh * half:(hh + 1) * half],
                            )
    nc.compile()
    return nc


def _host_prep(x, weight, bias):
    # zero-pad H and W by 1 on the host: border handling costs nothing here
    xp = np.pad(np.asarray(x, dtype=np.float32),
                ((0, 0), (0, 0), (1, 1), (1, 1)))
    xp = np.ascontiguousarray(xp)
    # weight OIHW -> [ci, (kh kw co)] so each lhsT tile is a contiguous slice
    w_host = np.ascontiguousarray(
        np.asarray(weight, dtype=np.float32).transpose(1, 2, 3, 0).reshape(C_IN, 9 * C_OUT)
    )
    # bias[co] -> [co % 128, co // 128]
    b_host = np.ascontiguousarray(
        np.asarray(bias, dtype=np.float32).reshape(N_CT, 128).T)
    return xp, w_host, b_host


def kernel(x, weight, bias, _trace=False):
    xp, w_host, b_host = _host_prep(x, weight, bias)
    nc = build_nc()
    in_maps = [
        {"x": xp[i * N_PER_CORE:(i + 1) * N_PER_CORE], "w": w_host, "b": b_host}
        for i in range(N_CORES)
    ]
    res = run_bass_kernel_spmd(nc, in_maps, core_ids=list(range(N_CORES)), trace=_trace)
    out = np.concatenate(
        [res.results[i]["out"].reshape(N_PER_CORE, C_OUT, H, W) for i in range(N_CORES)],
        axis=0,
    )
    if _trace:
        return out, res
    return out
